# revision 23
# baseline (speedup 1.0000x reference)
"""Multi-head self-attention (RoPE, causal) on 8 Trainium2 NeuronCores.

Sharding: core c -> (batch = c//2, head-group = c%2 of 8 heads).
Column-parallel wq/wk/wv, row-parallel wo. Each core emits a partial
out^T [f, s]; the host sums the two partials per batch and transposes.

Layouts (all chosen so no on-device transposes are needed):
  XT  [d, s]   (x transposed on host, bf16)
  Q^T/K^T [e, s] per head from matmul(lhsT=wT[d,e], rhs=XT[d,s])
  V   [s, e]   from matmul(lhsT=XT[d,s], rhs=wvT[d,e])
  S^T [j, i] = matmul(lhsT=K^T[e,j], rhs=Q^T[e,i])
  ctx^T [e, i] = matmul(lhsT=V[j,e], rhs=expS^T[j,i])
  out^T [f, s] = matmul(lhsT=woT[d,f], rhs=ctx^T[d,s])

All DRAM inputs/outputs are pre-tiled on the host into the exact
[128, N] blocks each DMA moves, so every DMA is a dense contiguous
copy. All matmul operands are bf16 (PSUM accumulation stays fp32);
softmax statistics and RoPE arithmetic stay fp32.

RoPE: pairs are laid out within 32-partition quadrants (host permutes
wq/wk rows): quadrant q holds even dims of freqs 16q..16q+15 in slots
0..15 and the matching odd dims in slots 16..31. The partner swap is a
single DVE stream_shuffle (no PE matmul); rot(x) = x*cc + shuf(x)*ss
with the sign folded into ss. The 1/sqrt(dk) scale is applied via the
Exp activation's scale field.

Softmax: no max-subtraction (scores are O(1)-scaled; fp32 exp is safe).
Causal masking by block-skipping + one 128x128 triangular mask on
diagonal blocks. Row sums via an all-ones [128,128] matmul (output rows
all equal the row sum, giving the partition broadcast for free);
normalization multiplies ctx^T by a fast DVE reciprocal of that tile.

Pipelining: each head's attention i-blocks interleave the next
projection chunk's matmuls (and V / next-head work) into the jt loop so
the PE never drains while the Scalar engine runs the exp chain.
"""

import numpy as np
import ml_dtypes

import concourse.bass as bass
import concourse.tile as tile
import concourse.mybir as mybir
from concourse import bacc, bass_utils

F32 = mybir.dt.float32
BF16 = mybir.dt.bfloat16

B = 4
S = 2048
D = 2048
NH = 16
DK = 128
NCORES = 8
HPC = 8            # heads per core
DLOC = HPC * DK    # 1024, local model dims per core
ST = S // 128      # 16 sequence 128-tiles
DT = D // 128      # 16 model-dim 128-tiles
NDT = DLOC // 128  # 8 local model-dim 128-tiles
IB = S // 512      # 4 i-blocks of 512
ROPE_THETA = 10000.0
SCALE = float(1.0 / np.sqrt(DK))
SHUF = list(range(16, 32)) + list(range(16))  # swap 16-halves per quadrant

_cache = {}


def build_program():
    if "nc" in _cache:
        return _cache["nc"]

    nc = bacc.Bacc("TRN2", target_bir_lowering=False, debug=False,
                   num_devices=NCORES)

    xt = nc.dram_tensor("xt", [DT, 4, 128, 512], BF16, kind="ExternalInput").ap()
    wq = nc.dram_tensor("wq", [HPC, DT, 128, DK], BF16, kind="ExternalInput").ap()
    wk = nc.dram_tensor("wk", [HPC, DT, 128, DK], BF16, kind="ExternalInput").ap()
    wv = nc.dram_tensor("wv", [2, DT, 128, 512], BF16, kind="ExternalInput").ap()
    wo = nc.dram_tensor("wo", [NDT, 128, D], BF16, kind="ExternalInput").ap()
    cct = nc.dram_tensor("cct", [128, S], F32, kind="ExternalInput").ap()
    sst = nc.dram_tensor("sst", [128, S], F32, kind="ExternalInput").ap()
    tri = nc.dram_tensor("tri", [128, 128], BF16, kind="ExternalInput").ap()
    out = nc.dram_tensor("out", [DT, IB, 128, 512], F32,
                         kind="ExternalOutput").ap()

    with tile.TileContext(nc) as tc:
        with (
            tc.tile_pool(name="dram", bufs=1, space="DRAM") as dram_pool,
            tc.tile_pool(name="ctx7", bufs=4) as ctx7_pool,
        ):
            ctx_dram = dram_pool.tile([HPC, IB, 128, 512], BF16)
            ctx7 = _attention_phase(nc, tc, xt, wq, wk, wv, cct, sst,
                                    tri, ctx_dram, ctx7_pool)
            _output_phase(nc, tc, wo, ctx_dram, out, ctx7)

    nc.compile()
    _cache["nc"] = nc
    return nc


def _attention_phase(nc, tc, xt, wq, wk, wv, cct, sst, tri, ctx_dram,
                     ctx7_pool):
    with (
        tc.tile_pool(name="xt", bufs=1) as xt_pool,
        tc.tile_pool(name="vsb", bufs=1) as v_pool,
        tc.tile_pool(name="tabs", bufs=1) as tab_pool,
        tc.tile_pool(name="wqk", bufs=2) as wqk_pool,
        tc.tile_pool(name="rope", bufs=2) as raw_pool,
        tc.tile_pool(name="rqk", bufs=2) as rqk_pool,
        tc.tile_pool(name="qk_ps", bufs=2, space="PSUM") as qk_ps_pool,
        tc.tile_pool(name="s_ps", bufs=3, space="PSUM") as s_ps_pool,
    ):
        # ---- resident tiles ----
        xt_sb = xt_pool.tile([128, DT, S], BF16)
        wv_sb = tab_pool.tile([128, DT, DLOC], BF16, tag="wv")
        cc_sb = tab_pool.tile([128, S], F32, tag="cct")
        ss_sb = tab_pool.tile([128, S], F32, tag="sst")
        tri_sb = tab_pool.tile([128, 128], BF16, tag="tri")
        ones_sb = tab_pool.tile([128, 128], BF16, tag="ones")

        def load_wqk(h):
            """Issue head h's wq/wk DMAs."""
            wq_sb = wqk_pool.tile([128, DT, DK], BF16, tag="wq")
            wk_sb = wqk_pool.tile([128, DT, DK], BF16, tag="wk")
            nc.sync.dma_start(wk_sb[:], wk[h].rearrange("d p k -> p d k",
                                                        p=128))
            nc.sync.dma_start(wq_sb[:], wq[h].rearrange("d p k -> p d k",
                                                        p=128))
            return wq_sb, wk_sb

        # ---- initial DMAs: earliest-consumed first; cc/ss/tri go on the
        # Scalar DMA queue in parallel with the Sync queue
        nc.scalar.dma_start(tri_sb[:], tri)
        nc.gpsimd.memset(ones_sb[:], 1.0)
        wq0_sb = wqk_pool.tile([128, DT, DK], BF16, tag="wq")
        wk0_sb = wqk_pool.tile([128, DT, DK], BF16, tag="wk")
        for g in range(0, DT, 4):
            nc.scalar.dma_start(
                wk0_sb[:, g:g + 4, :],
                wk[0, g:g + 4].rearrange("d p k -> p d k", p=128))
            nc.sync.dma_start(
                xt_sb[:, g:g + 4, 0:512],
                xt[g:g + 4, 0].rearrange("d p c -> p d c", p=128))
        nc.scalar.dma_start(cc_sb[:, 0:512], cct[:, 0:512])
        nc.scalar.dma_start(ss_sb[:, 0:512], sst[:, 0:512])
        nc.scalar.dma_start(cc_sb[:, 512:S], cct[:, 512:S])
        nc.scalar.dma_start(ss_sb[:, 512:S], sst[:, 512:S])
        for g in range(0, DT, 4):
            nc.sync.dma_start(
                wq0_sb[:, g:g + 4, :],
                wq[0, g:g + 4].rearrange("d p k -> p d k", p=128))
        for g in range(0, DT, 4):
            nc.sync.dma_start(
                wv_sb[:, g:g + 4, 0:512],
                wv[0, g:g + 4].rearrange("d p c -> p d c", p=128))
        for ch in range(1, 4):
            o = ch * 512
            for g in range(0, DT, 4):
                nc.sync.dma_start(
                    xt_sb[:, g:g + 4, o:o + 512],
                    xt[g:g + 4, ch].rearrange("d p c -> p d c", p=128))
        nc.sync.dma_start(wv_sb[:, :, 512:1024],
                          wv[1].rearrange("d p c -> p d c", p=128))

        def proj_thunks(w_sb, r_t, ch):
            """16 matmul thunks + 1 DVE rope-finish thunk for one chunk."""
            o = ch * 512
            ps = qk_ps_pool.tile([128, 512], F32, tag="qk_ps")
            thunks = [
                (lambda dt=dt: nc.tensor.matmul(
                    ps[:], w_sb[:, dt, :], xt_sb[:, dt, o:o + 512],
                    start=(dt == 0), stop=(dt == DT - 1)))
                for dt in range(DT)
            ]

            def rope():
                t3 = raw_pool.tile([128, 512], F32, tag="t3")
                nc.vector.tensor_mul(t3[:], ps[:], cc_sb[:, o:o + 512])
                swp = raw_pool.tile([128, 512], F32, tag="swp")
                nc.vector.stream_shuffle(swp[:], ps[:], SHUF)
                nc.vector.tensor_mul(swp[:], swp[:], ss_sb[:, o:o + 512])
                nc.vector.tensor_add(r_t[:, o:o + 512], t3[:], swp[:])

            thunks.append(rope)
            return thunks

        # ---- V = x @ wv.T (emitted at i-block starts, just in time) ----
        v_sb = v_pool.tile([128, ST, DLOC], BF16)

        def emit_v(st, g):
            v_ps = qk_ps_pool.tile([128, 512], F32, tag="qk_ps")
            for dt in range(DT):
                nc.tensor.matmul(
                    v_ps[:],
                    xt_sb[:, dt, st * 128:(st + 1) * 128],
                    wv_sb[:, dt, g * 512:(g + 1) * 512],
                    start=(dt == 0), stop=(dt == DT - 1),
                )
            nc.scalar.copy(v_sb[:, st, g * 512:(g + 1) * 512], v_ps[:])

        # ---- per-head attention with interleaved fill work ----
        with (
            tc.tile_pool(name="exps", bufs=6) as exp_pool,
            tc.tile_pool(name="pair", bufs=2) as pair_pool,
            tc.tile_pool(name="small", bufs=2) as small_pool,
            tc.tile_pool(name="ctxsb", bufs=2) as ctx_sb_pool,
            tc.tile_pool(name="ctx_ps", bufs=2, space="PSUM") as ctx_ps_pool,
            tc.tile_pool(name="rs_ps", bufs=1, space="PSUM") as rs_ps_pool,
        ):
            def run_ib(h, ib, rq, rk, fill):
                """Emit one i-block; drain `fill` thunks between jts."""
                i0 = ib * 512
                njt = 4 * ib + 4
                ctx_ps = ctx_ps_pool.tile([128, 512], F32, tag="ctx_ps")
                rs_ps = rs_ps_pool.tile([128, 512], F32, tag="rs_ps")
                es_prev = None
                quad = None
                nfill = len(fill)
                drained = 0
                for jt in range(njt):
                    r = jt - 4 * ib  # >=0 on diagonal blocks
                    lo = 128 * r if r >= 0 else 0
                    s_ps = s_ps_pool.tile([128, 512], F32, tag="s_ps")
                    nc.tensor.matmul(
                        s_ps[:, lo:512],
                        rk[:, jt * 128:(jt + 1) * 128],
                        rq[:, i0 + lo:i0 + 512],
                        start=True, stop=True,
                    )
                    es = exp_pool.tile([128, 512], BF16, tag="exps")
                    nc.scalar.activation(es[:, lo:512], s_ps[:, lo:512],
                                         mybir.ActivationFunctionType.Exp,
                                         scale=SCALE)
                    # drain an even share of fill thunks (PE work that is
                    # independent of this jt's exp) to cover the Scalar chain
                    want = (jt + 1) * nfill // njt
                    while drained < want:
                        fill[drained]()
                        drained += 1
                    if r >= 0:
                        nc.vector.tensor_mul(es[:, lo:lo + 128],
                                             es[:, lo:lo + 128], tri_sb[:])
                    first = (jt == 0)
                    last = (jt == njt - 1)
                    # row sums: full (off-diagonal) tiles come in groups of
                    # 4; tree-sum each quad on DVE and quarter the RS matmuls
                    if r < 0 and jt % 2 == 0:
                        es_prev = es
                    elif r < 0 and jt % 4 == 1:
                        quad = pair_pool.tile([128, 512], BF16, tag="pair")
                        nc.vector.tensor_add(quad[:], es_prev[:], es[:])
                    elif r < 0:
                        pair = pair_pool.tile([128, 512], BF16, tag="pair")
                        nc.vector.tensor_add(pair[:], es_prev[:], es[:])
                        nc.vector.tensor_add(quad[:], quad[:], pair[:])
                        if jt != 4 * ib - 1:
                            nc.tensor.matmul(
                                rs_ps[:],
                                ones_sb[:],
                                quad[:],
                                start=(jt == 3), stop=False,
                                skip_group_check=True,
                            )
                        # else: defer the last quad's RS; the fully-valid
                        # r=0 diagonal tile folds into it below
                    elif r == 0 and ib > 0:
                        nc.vector.tensor_add(quad[:], quad[:], es[:])
                        nc.tensor.matmul(
                            rs_ps[:],
                            ones_sb[:],
                            quad[:],
                            start=(ib == 1), stop=False,
                            skip_group_check=True,
                        )
                    else:
                        nc.tensor.matmul(
                            rs_ps[:, lo:512],
                            ones_sb[:],
                            es[:, lo:512],
                            start=first, stop=last, skip_group_check=True,
                        )
                    nc.tensor.matmul(
                        ctx_ps[:, lo:512],
                        v_sb[:, jt, h * DK:(h + 1) * DK],
                        es[:, lo:512],
                        start=first, stop=last, skip_group_check=True,
                    )
                recip = small_pool.tile([128, 512], F32, tag="recip")
                nc.vector.reciprocal_approx_fast(recip[:], rs_ps[:])
                if h == HPC - 1:
                    ctx_sb = ctx7_pool.tile([128, 512], BF16, tag="c7")
                    ctx7.append(ctx_sb)
                else:
                    ctx_sb = ctx_sb_pool.tile([128, 512], BF16, tag="ctx_sb")
                nc.vector.tensor_mul(ctx_sb[:], ctx_ps[:], recip[:])
                if h != HPC - 1:
                    nc.sync.dma_start(ctx_dram[h, ib], ctx_sb[:])

            ctx7 = []
            wqk = {0: (wq0_sb, wk0_sb)}
            rqk = {}
            for h in range(HPC):
                wq_sb, wk_sb = wqk.pop(h)
                if h == 0:
                    rq = rqk_pool.tile([128, S], BF16, tag="rq")
                    rk = rqk_pool.tile([128, S], BF16, tag="rk")
                    rqk[0] = (rq, rk)
                    for t in proj_thunks(wk_sb, rk, 0):
                        t()
                    for t in proj_thunks(wq_sb, rq, 0):
                        t()
                rq, rk = rqk.pop(h)
                if h + 1 < HPC:
                    # issue next head's weight DMAs well ahead of use
                    wqk[h + 1] = load_wqk(h + 1)
                    rq_n = rqk_pool.tile([128, S], BF16, tag="rq")
                    rk_n = rqk_pool.tile([128, S], BF16, tag="rk")
                    rqk[h + 1] = (rq_n, rk_n)
                for ib in range(IB):
                    # V tiles this i-block first needs, just in time
                    if h == 0:
                        for st in range(4 * ib, 4 * ib + 4):
                            emit_v(st, 0)
                    elif h == 1:
                        for st in range(4 * ib, 4 * ib + 4):
                            emit_v(st, 1)
                    fill = []
                    if ib < 3:
                        ch = ib + 1
                        fill += proj_thunks(wk_sb, rk, ch)
                        fill += proj_thunks(wq_sb, rq, ch)
                    elif h + 1 < HPC:
                        nwq, nwk = wqk[h + 1]
                        nrq, nrk = rqk[h + 1]
                        fill += proj_thunks(nwk, nrk, 0)
                        fill += proj_thunks(nwq, nrq, 0)
                    run_ib(h, ib, rq, rk, fill)
            return ctx7


def _output_phase(nc, tc, wo, ctx_dram, out, ctx7):
    with (
        tc.tile_pool(name="wos", bufs=1) as wo_pool,
        tc.tile_pool(name="ctxin", bufs=2) as cin_pool,
        tc.tile_pool(name="outsb", bufs=6) as out_pool,
        tc.tile_pool(name="wo_ps", bufs=8, space="PSUM") as wo_ps_pool,
    ):
        # all out-phase loads go on the Sync queue: it is idle well before
        # the attention tail (h7's ctx never round-trips), while the Scalar
        # queue is still draining h7's exps
        wo_sb = wo_pool.tile([128, NDT, D], BF16)
        nc.sync.dma_start(
            wo_sb[:, 0:2, :],
            wo[0:2].rearrange("d p f -> p d f", p=128))
        cins = []
        cin0 = cin_pool.tile([128, NDT - 1, 512], BF16, tag="cin")
        nc.sync.dma_start(
            cin0[:],
            ctx_dram[0:NDT - 1, 0].rearrange("h p c -> p h c", p=128))
        cins.append(cin0)
        for g in range(2, NDT, 2):
            nc.sync.dma_start(
                wo_sb[:, g:g + 2, :],
                wo[g:g + 2].rearrange("d p f -> p d f", p=128))
        # osb output DMAs alternate between the Sync and Scalar DMA queues
        # so the final flush isn't serialized on one queue
        out_qs = [nc.sync, nc.scalar]
        for sb4 in range(IB):
            if sb4 > 0:
                cin = cin_pool.tile([128, NDT - 1, 512], BF16, tag="cin")
                nc.sync.dma_start(
                    cin[:],
                    ctx_dram[0:NDT - 1, sb4].rearrange("h p c -> p h c",
                                                       p=128))
                cins.append(cin)
            cin = cins[sb4]
            # head 7's contribution comes from SBUF-resident ctx (no DRAM
            # round-trip on the tail); it is the last accumulation per group.
            # ft handled in groups (dt-major) so the first matmuls only need
            # the first wo/cin pieces; the last i-block uses smaller groups
            # so the final copy+DMA flush after the last matmul stays short.
            fgw = 4 if sb4 == IB - 1 else 8
            for fg in range(0, DT, fgw):
                pss = [wo_ps_pool.tile([128, 512], F32, tag="wo_ps",
                                       name="wo_ps")
                       for _ in range(fgw)]
                for dt in range(NDT - 1):
                    for fi in range(fgw):
                        ft = fg + fi
                        nc.tensor.matmul(
                            pss[fi][:],
                            wo_sb[:, dt, ft * 128:(ft + 1) * 128],
                            cin[:, dt, :],
                            start=(dt == 0), stop=False,
                            skip_group_check=True,
                        )
                for fi in range(fgw):
                    ft = fg + fi
                    nc.tensor.matmul(
                        pss[fi][:],
                        wo_sb[:, NDT - 1, ft * 128:(ft + 1) * 128],
                        ctx7[sb4][:],
                        start=False, stop=True,
                        skip_group_check=True,
                    )
                    osb = out_pool.tile([128, 512], F32, tag="osb")
                    nc.vector.tensor_copy(osb[:], pss[fi][:])
                    out_qs[ft % 2].dma_start(out[ft, sb4], osb[:])


def _tile2(a, p, q):
    """[R, C] -> [R//p, C//q, p, q] contiguous blocks."""
    R, C = a.shape
    return np.ascontiguousarray(
        a.reshape(R // p, p, C // q, q).transpose(0, 2, 1, 3))


def prepare_in_maps(x, wq, wk, wv, wo):
    """Build the 8 per-core input maps (host-side sharding + tables)."""
    x = np.asarray(x, dtype=np.float32)
    wq = np.asarray(wq, dtype=np.float32)
    wk = np.asarray(wk, dtype=np.float32)
    wv = np.asarray(wv, dtype=np.float32)
    wo = np.asarray(wo, dtype=np.float32)
    bf16 = ml_dtypes.bfloat16

    # RoPE tables (fp32, matching the reference's fp32 cos/sin).
    # Partition layout per head: quadrant qd (32 rows) holds even dims of
    # freqs 16qd..16qd+15 in slots 0..15 (cc=cos, ss=-sin) and the matching
    # odd dims in slots 16..31 (cc=cos, ss=+sin); stream_shuffle swaps the
    # 16-slot halves of each quadrant to pair partners.
    f = np.arange(0, DK, 2, dtype=np.float32) / DK          # 2f/d
    inv_freq = (ROPE_THETA ** (-f)).astype(np.float32)      # [64]
    ang = np.arange(S, dtype=np.float32)[:, None] * inv_freq[None, :]
    cos_t = np.cos(ang).T.astype(np.float32)                # [64, S]
    sin_t = np.sin(ang).T.astype(np.float32)
    perm = []
    f_of_p = np.empty(128, dtype=np.int64)
    sgn = np.empty(128, dtype=np.float32)
    for qd in range(4):
        for i in range(16):
            perm.append(2 * (16 * qd + i))
            f_of_p[32 * qd + i] = 16 * qd + i
            sgn[32 * qd + i] = -1.0
        for i in range(16):
            perm.append(2 * (16 * qd + i) + 1)
            f_of_p[32 * qd + 16 + i] = 16 * qd + i
            sgn[32 * qd + 16 + i] = 1.0
    deint = np.array(perm)
    cc = np.ascontiguousarray(cos_t[f_of_p, :])             # [128, S]
    ss = np.ascontiguousarray(sin_t[f_of_p, :] * sgn[:, None])

    tri = np.tril(np.ones((128, 128), dtype=np.float32)).T  # tri[j,i]=1 if j<=i
    tri = np.ascontiguousarray(tri).astype(bf16)

    in_maps = []
    for c in range(NCORES):
        bi, g = divmod(c, 2)
        heads = [g * HPC + h for h in range(HPC)]
        qk_rows = np.concatenate([hg * DK + deint for hg in heads])
        v_rows = np.arange(g * DLOC, (g + 1) * DLOC)

        xt_t = _tile2(x[bi].T.astype(bf16), 128, 512)            # [DT,4,128,512]
        # w*_prep [d, e_loc] -> [DT, HPC, 128, DK] -> [HPC, DT, 128, DK]
        wq_t = _tile2(wq[qk_rows, :].T.astype(bf16), 128, DK).transpose(1, 0, 2, 3)
        wk_t = _tile2(wk[qk_rows, :].T.astype(bf16), 128, DK).transpose(1, 0, 2, 3)
        # wv pre-tiled g-major: [2, DT, 128, 512]
        wv_t = _tile2(wv[v_rows, :].T.astype(bf16), 128, 512).transpose(1, 0, 2, 3)
        wo_t = _tile2(wo.T[v_rows, :].astype(bf16), 128, D)[:, 0]     # [NDT,128,D]
        in_maps.append({
            "xt": np.ascontiguousarray(xt_t),
            "wq": np.ascontiguousarray(wq_t),
            "wk": np.ascontiguousarray(wk_t),
            "wv": np.ascontiguousarray(wv_t),
            "wo": np.ascontiguousarray(wo_t),
            "cct": cc, "sst": ss,
            "tri": tri,
        })
    return in_maps


def assemble(results):
    out = np.empty((B, S, D), dtype=np.float32)
    for bi in range(B):
        oT = results[2 * bi]["out"] + results[2 * bi + 1]["out"]
        # oT: [DT, IB, 128, 512] -> out^T [f, s]; out[b] = out^T.T
        oT = oT.transpose(0, 2, 1, 3).reshape(D, S)
        out[bi] = oT.T
    return out


def kernel(**inputs):
    nc = build_program()
    in_maps = prepare_in_maps(inputs["x"], inputs["wq"], inputs["wk"],
                              inputs["wv"], inputs["wo"])
    res = bass_utils.run_bass_kernel_spmd(nc, in_maps,
                                          core_ids=list(range(NCORES)))
    return assemble(res.results)


# revision 25
# speedup vs baseline: 1.0153x; 1.0153x over previous
"""Multi-head self-attention (RoPE, causal) on 8 Trainium2 NeuronCores.

Sharding: core c -> (batch = c//2, head-group = c%2 of 8 heads).
Column-parallel wq/wk/wv, row-parallel wo. Each core emits a partial
out^T [f, s]; the host sums the two partials per batch and transposes.

Layouts (all chosen so no on-device transposes are needed):
  XT  [d, s]   (x transposed on host, bf16)
  Q^T/K^T [e, s] per head from matmul(lhsT=wT[d,e], rhs=XT[d,s])
  V   [s, e]   from matmul(lhsT=XT[d,s], rhs=wvT[d,e])
  S^T [j, i] = matmul(lhsT=K^T[e,j], rhs=Q^T[e,i])
  ctx^T [e, i] = matmul(lhsT=V[j,e], rhs=expS^T[j,i])
  out^T [f, s] = matmul(lhsT=woT[d,f], rhs=ctx^T[d,s])

All DRAM inputs/outputs are pre-tiled on the host into the exact
[128, N] blocks each DMA moves, so every DMA is a dense contiguous
copy. All matmul operands are bf16 (PSUM accumulation stays fp32);
softmax statistics and RoPE arithmetic stay fp32.

RoPE: pairs are laid out within 32-partition quadrants (host permutes
wq/wk rows): quadrant q holds even dims of freqs 16q..16q+15 in slots
0..15 and the matching odd dims in slots 16..31. The partner swap is a
single DVE stream_shuffle (no PE matmul); rot(x) = x*cc + shuf(x)*ss
with the sign folded into ss. The 1/sqrt(dk) scale is applied via the
Exp activation's scale field.

Softmax: no max-subtraction (scores are O(1)-scaled; fp32 exp is safe).
Causal masking by block-skipping + one 128x128 triangular mask on
diagonal blocks. Row sums via an all-ones [128,128] matmul (output rows
all equal the row sum, giving the partition broadcast for free);
normalization multiplies ctx^T by a fast DVE reciprocal of that tile.

Pipelining: each head's attention i-blocks interleave the next
projection chunk's matmuls (and V / next-head work) into the jt loop so
the PE never drains while the Scalar engine runs the exp chain.
"""

import numpy as np
import ml_dtypes

import concourse.bass as bass
import concourse.tile as tile
import concourse.mybir as mybir
from concourse import bacc, bass_utils

F32 = mybir.dt.float32
BF16 = mybir.dt.bfloat16

B = 4
S = 2048
D = 2048
NH = 16
DK = 128
NCORES = 8
HPC = 8            # heads per core
DLOC = HPC * DK    # 1024, local model dims per core
ST = S // 128      # 16 sequence 128-tiles
DT = D // 128      # 16 model-dim 128-tiles
NDT = DLOC // 128  # 8 local model-dim 128-tiles
IB = S // 512      # 4 i-blocks of 512
ROPE_THETA = 10000.0
SCALE = float(1.0 / np.sqrt(DK))
SHUF = list(range(16, 32)) + list(range(16))  # swap 16-halves per quadrant

_cache = {}


def build_program():
    if "nc" in _cache:
        return _cache["nc"]

    nc = bacc.Bacc("TRN2", target_bir_lowering=False, debug=False,
                   num_devices=NCORES)

    xt = nc.dram_tensor("xt", [DT, 4, 128, 512], BF16, kind="ExternalInput").ap()
    wq = nc.dram_tensor("wq", [HPC, DT, 128, DK], BF16, kind="ExternalInput").ap()
    wk = nc.dram_tensor("wk", [HPC, DT, 128, DK], BF16, kind="ExternalInput").ap()
    wv = nc.dram_tensor("wv", [2, DT, 128, 512], BF16, kind="ExternalInput").ap()
    wo = nc.dram_tensor("wo", [NDT, 128, D], BF16, kind="ExternalInput").ap()
    cct = nc.dram_tensor("cct", [128, S], F32, kind="ExternalInput").ap()
    sst = nc.dram_tensor("sst", [128, S], F32, kind="ExternalInput").ap()
    tri = nc.dram_tensor("tri", [128, 128], BF16, kind="ExternalInput").ap()
    out = nc.dram_tensor("out", [DT, IB, 128, 512], F32,
                         kind="ExternalOutput").ap()

    with tile.TileContext(nc) as tc:
        with (
            tc.tile_pool(name="dram", bufs=1, space="DRAM") as dram_pool,
            tc.tile_pool(name="ctx7", bufs=4) as ctx7_pool,
        ):
            ctx_dram = dram_pool.tile([HPC, IB, 128, 512], BF16)
            ctx7 = _attention_phase(nc, tc, xt, wq, wk, wv, cct, sst,
                                    tri, ctx_dram, ctx7_pool)
            _output_phase(nc, tc, wo, ctx_dram, out, ctx7)

    nc.compile()
    _cache["nc"] = nc
    return nc


def _attention_phase(nc, tc, xt, wq, wk, wv, cct, sst, tri, ctx_dram,
                     ctx7_pool):
    with (
        tc.tile_pool(name="xt", bufs=1) as xt_pool,
        tc.tile_pool(name="vsb", bufs=1) as v_pool,
        tc.tile_pool(name="tabs", bufs=1) as tab_pool,
        tc.tile_pool(name="wqk", bufs=2) as wqk_pool,
        tc.tile_pool(name="rope", bufs=2) as raw_pool,
        tc.tile_pool(name="rqk", bufs=2) as rqk_pool,
        tc.tile_pool(name="qk_ps", bufs=2, space="PSUM") as qk_ps_pool,
        tc.tile_pool(name="s_ps", bufs=3, space="PSUM") as s_ps_pool,
    ):
        # ---- resident tiles ----
        xt_sb = xt_pool.tile([128, DT, S], BF16)
        wv_sb = tab_pool.tile([128, DT, DLOC], BF16, tag="wv")
        cc_sb = tab_pool.tile([128, S], F32, tag="cct")
        ss_sb = tab_pool.tile([128, S], F32, tag="sst")
        tri_sb = tab_pool.tile([128, 128], BF16, tag="tri")
        ones_sb = tab_pool.tile([128, 128], BF16, tag="ones")

        def load_wqk(h):
            """Issue head h's wq/wk DMAs."""
            wq_sb = wqk_pool.tile([128, DT, DK], BF16, tag="wq")
            wk_sb = wqk_pool.tile([128, DT, DK], BF16, tag="wk")
            nc.sync.dma_start(wk_sb[:], wk[h].rearrange("d p k -> p d k",
                                                        p=128))
            nc.sync.dma_start(wq_sb[:], wq[h].rearrange("d p k -> p d k",
                                                        p=128))
            return wq_sb, wk_sb

        # ---- initial DMAs: earliest-consumed first; cc/ss/tri go on the
        # Scalar DMA queue in parallel with the Sync queue
        nc.scalar.dma_start(tri_sb[:], tri)
        nc.gpsimd.memset(ones_sb[:], 1.0)
        wq0_sb = wqk_pool.tile([128, DT, DK], BF16, tag="wq")
        wk0_sb = wqk_pool.tile([128, DT, DK], BF16, tag="wk")
        for g in range(0, DT, 4):
            nc.scalar.dma_start(
                wk0_sb[:, g:g + 4, :],
                wk[0, g:g + 4].rearrange("d p k -> p d k", p=128))
            nc.sync.dma_start(
                xt_sb[:, g:g + 4, 0:512],
                xt[g:g + 4, 0].rearrange("d p c -> p d c", p=128))
        nc.scalar.dma_start(cc_sb[:, 0:512], cct[:, 0:512])
        nc.scalar.dma_start(ss_sb[:, 0:512], sst[:, 0:512])
        nc.scalar.dma_start(cc_sb[:, 512:S], cct[:, 512:S])
        nc.scalar.dma_start(ss_sb[:, 512:S], sst[:, 512:S])
        for g in range(0, DT, 4):
            nc.sync.dma_start(
                wq0_sb[:, g:g + 4, :],
                wq[0, g:g + 4].rearrange("d p k -> p d k", p=128))
        for g in range(0, DT, 4):
            nc.sync.dma_start(
                wv_sb[:, g:g + 4, 0:512],
                wv[0, g:g + 4].rearrange("d p c -> p d c", p=128))
        for ch in range(1, 4):
            o = ch * 512
            for g in range(0, DT, 4):
                nc.sync.dma_start(
                    xt_sb[:, g:g + 4, o:o + 512],
                    xt[g:g + 4, ch].rearrange("d p c -> p d c", p=128))
        nc.sync.dma_start(wv_sb[:, :, 512:1024],
                          wv[1].rearrange("d p c -> p d c", p=128))

        def proj_thunks(w_sb, r_t, ch):
            """16 matmul thunks + 1 DVE rope-finish thunk for one chunk."""
            o = ch * 512
            ps = qk_ps_pool.tile([128, 512], F32, tag="qk_ps")
            thunks = [
                (lambda dt=dt: nc.tensor.matmul(
                    ps[:], w_sb[:, dt, :], xt_sb[:, dt, o:o + 512],
                    start=(dt == 0), stop=(dt == DT - 1)))
                for dt in range(DT)
            ]

            def rope():
                t3 = raw_pool.tile([128, 512], F32, tag="t3")
                nc.vector.tensor_mul(t3[:], ps[:], cc_sb[:, o:o + 512])
                swp = raw_pool.tile([128, 512], F32, tag="swp")
                nc.vector.stream_shuffle(swp[:], ps[:], SHUF)
                nc.vector.tensor_mul(swp[:], swp[:], ss_sb[:, o:o + 512])
                nc.vector.tensor_add(r_t[:, o:o + 512], t3[:], swp[:])

            thunks.append(rope)
            return thunks

        # ---- V = x @ wv.T (emitted at i-block starts, just in time) ----
        v_sb = v_pool.tile([128, ST, DLOC], BF16)

        def emit_v(st, g):
            v_ps = qk_ps_pool.tile([128, 512], F32, tag="qk_ps")
            for dt in range(DT):
                nc.tensor.matmul(
                    v_ps[:],
                    xt_sb[:, dt, st * 128:(st + 1) * 128],
                    wv_sb[:, dt, g * 512:(g + 1) * 512],
                    start=(dt == 0), stop=(dt == DT - 1),
                )
            nc.scalar.copy(v_sb[:, st, g * 512:(g + 1) * 512], v_ps[:])

        # ---- per-head attention with interleaved fill work ----
        with (
            tc.tile_pool(name="exps", bufs=6) as exp_pool,
            tc.tile_pool(name="pair", bufs=2) as pair_pool,
            tc.tile_pool(name="small", bufs=2) as small_pool,
            tc.tile_pool(name="ctxsb", bufs=2) as ctx_sb_pool,
            tc.tile_pool(name="ctx_ps", bufs=2, space="PSUM") as ctx_ps_pool,
            tc.tile_pool(name="rs_ps", bufs=1, space="PSUM") as rs_ps_pool,
        ):
            def run_ib(h, ib, rq, rk, fill):
                """Emit one i-block; drain `fill` thunks between jts."""
                i0 = ib * 512
                njt = 4 * ib + 4
                ctx_ps = ctx_ps_pool.tile([128, 512], F32, tag="ctx_ps")
                rs_ps = rs_ps_pool.tile([128, 512], F32, tag="rs_ps")
                es_prev = None
                quad = None
                nfill = len(fill)
                drained = 0
                for jt in range(njt):
                    r = jt - 4 * ib  # >=0 on diagonal blocks
                    lo = 128 * r if r >= 0 else 0
                    s_ps = s_ps_pool.tile([128, 512], F32, tag="s_ps")
                    nc.tensor.matmul(
                        s_ps[:, lo:512],
                        rk[:, jt * 128:(jt + 1) * 128],
                        rq[:, i0 + lo:i0 + 512],
                        start=True, stop=True,
                    )
                    es = exp_pool.tile([128, 512], BF16, tag="exps")
                    nc.scalar.activation(es[:, lo:512], s_ps[:, lo:512],
                                         mybir.ActivationFunctionType.Exp,
                                         scale=SCALE)
                    # drain an even share of fill thunks (PE work that is
                    # independent of this jt's exp) to cover the Scalar chain
                    want = (jt + 1) * nfill // njt
                    while drained < want:
                        fill[drained]()
                        drained += 1
                    if r >= 0:
                        nc.vector.tensor_mul(es[:, lo:lo + 128],
                                             es[:, lo:lo + 128], tri_sb[:])
                    first = (jt == 0)
                    last = (jt == njt - 1)
                    # row sums: full (off-diagonal) tiles come in groups of
                    # 4; tree-sum each quad on DVE and quarter the RS matmuls
                    if r < 0 and jt % 2 == 0:
                        es_prev = es
                    elif r < 0 and jt % 4 == 1:
                        quad = pair_pool.tile([128, 512], BF16, tag="pair")
                        nc.vector.tensor_add(quad[:], es_prev[:], es[:])
                    elif r < 0:
                        pair = pair_pool.tile([128, 512], BF16, tag="pair")
                        nc.vector.tensor_add(pair[:], es_prev[:], es[:])
                        nc.vector.tensor_add(quad[:], quad[:], pair[:])
                        if jt != 4 * ib - 1:
                            nc.tensor.matmul(
                                rs_ps[:],
                                ones_sb[:],
                                quad[:],
                                start=(jt == 3), stop=False,
                                skip_group_check=True,
                            )
                        # else: defer the last quad's RS; the fully-valid
                        # r=0 diagonal tile folds into it below
                    elif r == 0 and ib > 0:
                        nc.vector.tensor_add(quad[:], quad[:], es[:])
                        nc.tensor.matmul(
                            rs_ps[:],
                            ones_sb[:],
                            quad[:],
                            start=(ib == 1), stop=False,
                            skip_group_check=True,
                        )
                    else:
                        nc.tensor.matmul(
                            rs_ps[:, lo:512],
                            ones_sb[:],
                            es[:, lo:512],
                            start=first, stop=last, skip_group_check=True,
                        )
                    nc.tensor.matmul(
                        ctx_ps[:, lo:512],
                        v_sb[:, jt, h * DK:(h + 1) * DK],
                        es[:, lo:512],
                        start=first, stop=last, skip_group_check=True,
                    )
                recip = small_pool.tile([128, 512], F32, tag="recip")
                nc.vector.reciprocal_approx_fast(recip[:], rs_ps[:])
                if h == HPC - 1:
                    ctx_sb = ctx7_pool.tile([128, 512], BF16, tag="c7")
                    ctx7.append(ctx_sb)
                else:
                    ctx_sb = ctx_sb_pool.tile([128, 512], BF16, tag="ctx_sb")
                nc.vector.tensor_mul(ctx_sb[:], ctx_ps[:], recip[:])
                if h != HPC - 1:
                    nc.sync.dma_start(ctx_dram[h, ib], ctx_sb[:])

            ctx7 = []
            wqk = {0: (wq0_sb, wk0_sb)}
            rqk = {}
            for h in range(HPC):
                wq_sb, wk_sb = wqk.pop(h)
                if h == 0:
                    rq = rqk_pool.tile([128, S], BF16, tag="rq")
                    rk = rqk_pool.tile([128, S], BF16, tag="rk")
                    rqk[0] = (rq, rk)
                    for t in proj_thunks(wk_sb, rk, 0):
                        t()
                    for t in proj_thunks(wq_sb, rq, 0):
                        t()
                rq, rk = rqk.pop(h)
                if h + 1 < HPC:
                    # issue next head's weight DMAs well ahead of use
                    wqk[h + 1] = load_wqk(h + 1)
                    rq_n = rqk_pool.tile([128, S], BF16, tag="rq")
                    rk_n = rqk_pool.tile([128, S], BF16, tag="rk")
                    rqk[h + 1] = (rq_n, rk_n)
                for ib in range(IB):
                    # V tiles this i-block first needs, just in time
                    if h == 0:
                        for st in range(4 * ib, 4 * ib + 4):
                            emit_v(st, 0)
                    elif h == 1:
                        for st in range(4 * ib, 4 * ib + 4):
                            emit_v(st, 1)
                    fill = []
                    if ib < 3:
                        ch = ib + 1
                        fill += proj_thunks(wk_sb, rk, ch)
                        fill += proj_thunks(wq_sb, rq, ch)
                    elif h + 1 < HPC:
                        nwq, nwk = wqk[h + 1]
                        nrq, nrk = rqk[h + 1]
                        fill += proj_thunks(nwk, nrk, 0)
                        fill += proj_thunks(nwq, nrq, 0)
                    run_ib(h, ib, rq, rk, fill)
            return ctx7


def _output_phase(nc, tc, wo, ctx_dram, out, ctx7):
    with (
        tc.tile_pool(name="wos", bufs=1) as wo_pool,
        tc.tile_pool(name="ctxin", bufs=4) as cin_pool,
        tc.tile_pool(name="outsb", bufs=6) as out_pool,
        tc.tile_pool(name="wo_ps", bufs=8, space="PSUM") as wo_ps_pool,
    ):
        # all out-phase loads go on the Sync queue: it is idle well before
        # the attention tail (h7's ctx never round-trips), while the Scalar
        # queue is still draining h7's exps
        wo_sb = wo_pool.tile([128, NDT, D], BF16)
        nc.sync.dma_start(
            wo_sb[:, 0:2, :],
            wo[0:2].rearrange("d p f -> p d f", p=128))
        # prefetch ALL cin blocks before any osb write enters the queues:
        # DMA queues are in-order, and osb writes wait on their copies, so a
        # cin issued behind them would head-of-line block until compute ends
        cins = []
        for sb4 in range(IB):
            cin = cin_pool.tile([128, NDT - 1, 512], BF16, tag="cin",
                                name="cin")
            nc.sync.dma_start(
                cin[:],
                ctx_dram[0:NDT - 1, sb4].rearrange("h p c -> p h c", p=128))
            cins.append(cin)
        for g in range(2, NDT, 2):
            nc.sync.dma_start(
                wo_sb[:, g:g + 2, :],
                wo[g:g + 2].rearrange("d p f -> p d f", p=128))
        # osb output DMAs alternate between the Sync and Scalar DMA queues
        # so the final flush isn't serialized on one queue
        out_qs = [nc.sync, nc.scalar]
        for sb4 in range(IB):
            cin = cins[sb4]
            # head 7's contribution comes from SBUF-resident ctx (no DRAM
            # round-trip on the tail); it is the last accumulation per group.
            # ft handled in groups (dt-major) so the first matmuls only need
            # the first wo/cin pieces; the last i-block uses smaller groups
            # so the final copy+DMA flush after the last matmul stays short.
            fgw = 4 if sb4 == IB - 1 else 8
            for fg in range(0, DT, fgw):
                pss = [wo_ps_pool.tile([128, 512], F32, tag="wo_ps",
                                       name="wo_ps")
                       for _ in range(fgw)]
                for dt in range(NDT - 1):
                    for fi in range(fgw):
                        ft = fg + fi
                        nc.tensor.matmul(
                            pss[fi][:],
                            wo_sb[:, dt, ft * 128:(ft + 1) * 128],
                            cin[:, dt, :],
                            start=(dt == 0), stop=False,
                            skip_group_check=True,
                        )
                for fi in range(fgw):
                    ft = fg + fi
                    nc.tensor.matmul(
                        pss[fi][:],
                        wo_sb[:, NDT - 1, ft * 128:(ft + 1) * 128],
                        ctx7[sb4][:],
                        start=False, stop=True,
                        skip_group_check=True,
                    )
                    osb = out_pool.tile([128, 512], F32, tag="osb")
                    nc.vector.tensor_copy(osb[:], pss[fi][:])
                    out_qs[ft % 2].dma_start(out[ft, sb4], osb[:])


def _tile2(a, p, q):
    """[R, C] -> [R//p, C//q, p, q] contiguous blocks."""
    R, C = a.shape
    return np.ascontiguousarray(
        a.reshape(R // p, p, C // q, q).transpose(0, 2, 1, 3))


def prepare_in_maps(x, wq, wk, wv, wo):
    """Build the 8 per-core input maps (host-side sharding + tables)."""
    x = np.asarray(x, dtype=np.float32)
    wq = np.asarray(wq, dtype=np.float32)
    wk = np.asarray(wk, dtype=np.float32)
    wv = np.asarray(wv, dtype=np.float32)
    wo = np.asarray(wo, dtype=np.float32)
    bf16 = ml_dtypes.bfloat16

    # RoPE tables (fp32, matching the reference's fp32 cos/sin).
    # Partition layout per head: quadrant qd (32 rows) holds even dims of
    # freqs 16qd..16qd+15 in slots 0..15 (cc=cos, ss=-sin) and the matching
    # odd dims in slots 16..31 (cc=cos, ss=+sin); stream_shuffle swaps the
    # 16-slot halves of each quadrant to pair partners.
    f = np.arange(0, DK, 2, dtype=np.float32) / DK          # 2f/d
    inv_freq = (ROPE_THETA ** (-f)).astype(np.float32)      # [64]
    ang = np.arange(S, dtype=np.float32)[:, None] * inv_freq[None, :]
    cos_t = np.cos(ang).T.astype(np.float32)                # [64, S]
    sin_t = np.sin(ang).T.astype(np.float32)
    perm = []
    f_of_p = np.empty(128, dtype=np.int64)
    sgn = np.empty(128, dtype=np.float32)
    for qd in range(4):
        for i in range(16):
            perm.append(2 * (16 * qd + i))
            f_of_p[32 * qd + i] = 16 * qd + i
            sgn[32 * qd + i] = -1.0
        for i in range(16):
            perm.append(2 * (16 * qd + i) + 1)
            f_of_p[32 * qd + 16 + i] = 16 * qd + i
            sgn[32 * qd + 16 + i] = 1.0
    deint = np.array(perm)
    cc = np.ascontiguousarray(cos_t[f_of_p, :])             # [128, S]
    ss = np.ascontiguousarray(sin_t[f_of_p, :] * sgn[:, None])

    tri = np.tril(np.ones((128, 128), dtype=np.float32)).T  # tri[j,i]=1 if j<=i
    tri = np.ascontiguousarray(tri).astype(bf16)

    in_maps = []
    for c in range(NCORES):
        bi, g = divmod(c, 2)
        heads = [g * HPC + h for h in range(HPC)]
        qk_rows = np.concatenate([hg * DK + deint for hg in heads])
        v_rows = np.arange(g * DLOC, (g + 1) * DLOC)

        xt_t = _tile2(x[bi].T.astype(bf16), 128, 512)            # [DT,4,128,512]
        # w*_prep [d, e_loc] -> [DT, HPC, 128, DK] -> [HPC, DT, 128, DK]
        wq_t = _tile2(wq[qk_rows, :].T.astype(bf16), 128, DK).transpose(1, 0, 2, 3)
        wk_t = _tile2(wk[qk_rows, :].T.astype(bf16), 128, DK).transpose(1, 0, 2, 3)
        # wv pre-tiled g-major: [2, DT, 128, 512]
        wv_t = _tile2(wv[v_rows, :].T.astype(bf16), 128, 512).transpose(1, 0, 2, 3)
        wo_t = _tile2(wo.T[v_rows, :].astype(bf16), 128, D)[:, 0]     # [NDT,128,D]
        in_maps.append({
            "xt": np.ascontiguousarray(xt_t),
            "wq": np.ascontiguousarray(wq_t),
            "wk": np.ascontiguousarray(wk_t),
            "wv": np.ascontiguousarray(wv_t),
            "wo": np.ascontiguousarray(wo_t),
            "cct": cc, "sst": ss,
            "tri": tri,
        })
    return in_maps


def assemble(results):
    out = np.empty((B, S, D), dtype=np.float32)
    for bi in range(B):
        oT = results[2 * bi]["out"] + results[2 * bi + 1]["out"]
        # oT: [DT, IB, 128, 512] -> out^T [f, s]; out[b] = out^T.T
        oT = oT.transpose(0, 2, 1, 3).reshape(D, S)
        out[bi] = oT.T
    return out


def kernel(**inputs):
    nc = build_program()
    in_maps = prepare_in_maps(inputs["x"], inputs["wq"], inputs["wk"],
                              inputs["wv"], inputs["wo"])
    res = bass_utils.run_bass_kernel_spmd(nc, in_maps,
                                          core_ids=list(range(NCORES)))
    return assemble(res.results)
